# revision 32
# baseline (speedup 1.0000x reference)
"""Trainium2 Bass kernel for GQA attention (S=2048, D=4096, 32 Q heads / 8 KV
heads, head_dim=128, RoPE, full softmax) tensor-parallel over 8 NeuronCores.

Sharding: Megatron TP. Core c gets Q heads 4c..4c+3 and KV head c
(column-shard wq/wk/wv, row-shard wo). The output all-reduce is done on host
(sum of 8 partial outputs).

Layout: every on-device matmul contracts over the partition dim, so the
projection is computed with *transposed* output (channels on partitions) by
feeding x^T (host-prepped). RoPE pairs are laid out so DVE stream_shuffle
(which permutes lanes within each 32-partition block) swaps t1/t2, making the
rotation 4 elementwise DVE ops per chunk. scores^T (k on partitions, q free)
feeds exp on ACT; exp tiles feed the V-matmul (V natural layout via one PE
transpose per 128-row tile) and a ones-column matmul accumulating the softmax
denominator. attn^T overwrites the consumed Q slice and feeds the wo matmul
as the stationary operand with no transposes. Output projection is
interleaved per q-chunk with the attention of the next chunk.

Scheduling notes (trace-driven):
- The steady-state N=512 fp16 matmul issue gap is ~218ns (streaming bound
  512/2.4GHz + NX overhead); the kernel keeps the in-order PE within ~3% of
  that by never letting a slow engine's chain sit in front of ready matmuls.
- Softmax normalization (ones-matmul -> reciprocal -> per-qt muls) for each
  head is emitted at kt0 of the NEXT head's k-loop; emitted in place it
  parks the PE behind the ACT/DVE exp-accumulate tail (~2-3us per incident,
  plus a HAM re-throttle to 1.2GHz after any >3.4us idle).
- wo/deferred-Q matmuls are PE filler pulled into the (ACT-paced) attention
  k-loops on a budget scheduler (1-3 pulls/kt) so no stretch of the k-loop
  is left filler-less and no leftover burst drains at qc boundaries.
- ACT must stay exp-only in phase 2: routing even part of the PSUM->SBUF
  staging copies to it convoys the exp pipeline (+20us measured).
- sc0 of the projection is DMA-bandwidth-bound (~10.3MB before its matmuls
  can run densely, HBM saturated at ~340GB/s): W rides 2-dt 393KB batched
  3D-AP transfers over three issue queues, sc0's x pairs get a full-depth
  pool while sc1's prefetch sits in a shallow pool so it cannot steal sc0
  bandwidth; 18 junk warmup matmuls cover the first-arrival latency and
  hold the PE HAM clock at 8/8.
- fp16 output + fp32 host reduce; fp8 was evaluated numerically and is
  unusable here (one fp8 matmul already gives 3-9% rel err vs the 2e-2
  gate, since per-element quantization noise passes ~1:1 to the output).
"""

import os
import sys

sys.path.insert(0, "/opt/trn_rl_repo")

import numpy as np

S = 2048
DIM = 4096
N_HEADS = 32
N_KV_HEADS = 8
HEAD_DIM = 128
N_CORES = 8
QH = N_HEADS // N_CORES          # 4 Q heads per core
QW = QH * HEAD_DIM               # 512 Q cols per core
NW = QW + 2 * HEAD_DIM           # 768 projection cols per core (q + k + v)
SCALE = HEAD_DIM ** -0.5
D_TILES = DIM // 128             # 32
S_TILES = S // 128               # 16
S_CHUNKS = S // 512              # 4
N_TILES = NW // 128              # 6
E_CHUNKS = DIM // 512            # 8

_cache = {}


def _build_program(debug=False):
    import concourse.mybir as mybir
    import concourse.tile as tile
    from concourse import bacc

    f32 = mybir.dt.float32
    f32r = mybir.dt.float32r
    f16 = mybir.dt.float16

    nc = bacc.Bacc("TRN2", target_bir_lowering=False, debug=False)

    xT = nc.dram_tensor("xT", [DIM, S], f16, kind="ExternalInput").ap()
    wqkv = nc.dram_tensor("wqkv", [DIM, NW], f16, kind="ExternalInput").ap()
    wo = nc.dram_tensor("wo", [QW, DIM], f16, kind="ExternalInput").ap()
    cos2 = nc.dram_tensor("cos2", [128, S], f16, kind="ExternalInput").ap()
    sinm2 = nc.dram_tensor("sinm2", [128, S], f16, kind="ExternalInput").ap()
    ident = nc.dram_tensor("ident", [128, 128], f16, kind="ExternalInput").ap()
    ones = nc.dram_tensor("ones", [128, 128], f16, kind="ExternalInput").ap()
    # fp16 output halves the output-DMA bytes; the host all-reduce sums in
    # fp32 (adds ~2e-4 rel err vs the 2e-2 gate)
    outp = nc.dram_tensor("outp", [S, DIM], f16, kind="ExternalOutput").ap()

    # stream_shuffle: out lane (32b+i) = in lane (32b + mask[i])
    swap_mask = [(i + 16) % 32 for i in range(32)]

    with tile.TileContext(nc) as tc:
        with (
            tc.tile_pool(name="persist", bufs=1) as persist,
            tc.tile_pool(name="vn", bufs=1) as vnp,
            tc.tile_pool(name="wpool", bufs=1) as wpool,
            tc.tile_pool(name="cspool", bufs=1) as cspool,
            tc.tile_pool(name="rtmp", bufs=2) as rtmp,
            tc.tile_pool(name="x3pool", bufs=D_TILES // 2) as x3pool,
        ):
            QKVT = persist.tile([128, N_TILES * S], f16, tag="qkvt")
            EBIAS = persist.tile([128, 1], f32, tag="ebias")
            nc.gpsimd.memset(EBIAS[:], -4.0)
            VN = vnp.tile([128, S], f16, tag="vn")
            ONES = persist.tile([128, 128], f16, tag="ones")

            W = wpool.tile([128, D_TILES * NW], f16, tag="w")
            COS2 = cspool.tile([128, S], f16, tag="cos2")
            SINM2 = cspool.tile([128, S], f16, tag="sinm2")
            IDENT = cspool.tile([128, 128], f16, tag="ident")

            def rope(h, sc):
                sl = slice(h * S + sc * 512, h * S + (sc + 1) * 512)
                csl = slice(sc * 512, (sc + 1) * 512)
                sh = rtmp.tile([128, 512], f16, tag="sh",
                               name=f"sh_{h}_{sc}")
                nc.vector.stream_shuffle(sh[:], QKVT[:, sl], swap_mask)
                nc.vector.tensor_mul(sh[:], sh[:], SINM2[:, csl])
                tm = rtmp.tile([128, 512], f16, tag="tm",
                               name=f"tm_{h}_{sc}")
                nc.vector.tensor_mul(tm[:], QKVT[:, sl], COS2[:, csl])
                nc.vector.tensor_add(QKVT[:, sl], sh[:], tm[:])

            # ---------------- Phase 1: fused QKV projection (transposed) ---
            with (
                tc.tile_pool(name="xpool", bufs=4) as xpool,
                tc.tile_pool(name="x0pool", bufs=16) as x0pool,
                tc.tile_pool(name="ppsum", bufs=7, space="PSUM") as ppsum,
                tc.tile_pool(name="tpsum", bufs=1, space="PSUM") as tpsum,
            ):

                # K (nt=4) and V (nt=5) first so RoPE(K)/transpose(V) start
                # early
                nt_order = [4, 5, 0, 1, 2, 3]

                # HAM warmup: the first real matmul can't start until its
                # W/x DMAs land (~11us); run junk matmuls on a memset scratch
                # region meanwhile so the PE clock is at 8/8 when real work
                # arrives (else the first ~3.4us run at half rate).
                # gpsimd clears its preamble ~1.4us before DVE does, so the
                # memset (and with it the warmup) starts that much earlier
                nc.gpsimd.memset(QKVT[:, 0:640], 0.0)
                wpt = ppsum.tile([128, 512], f32, tag="proj", name="warm")
                for _ in range(18):
                    nc.tensor.matmul(wpt[:], QKVT[:, 512:640],
                                     QKVT[:, 0:512],
                                     start=True, stop=True)

                def transpose_v(st):
                    pt = tpsum.tile([128, 128], f16, tag="vt",
                                    name=f"vt_{st}")
                    nc.tensor.transpose(
                        pt[:],
                        QKVT[:, 5 * S + st * 128: 5 * S + (st + 1) * 128],
                        IDENT[:],
                    )
                    nc.scalar.copy(VN[:, st * 128:(st + 1) * 128], pt[:])

                # each dma_start costs ~600ns of sequencer issue time, so
                # batch transfers: W rides in 4-dt groups (786KB contiguous
                # 1536B lines), x in 2-dt pairs (256KB) — halves the issue
                # count for sc=0, which must stream W 6.3MB + x 4MB before
                # its matmuls can run densely
                x3_tiles = []
                for sc in range(S_CHUNKS):
                    nts = nt_order if sc < S_CHUNKS - 1 else [4, 5, 0]
                    psums = {}
                    for dt in range(D_TILES):
                        if sc == 0 and dt % 2 == 0:
                            # three issue engines for sc0's W burst (ACT's
                            # DMAs finish long before its first copy_out);
                            # 2-dt groups (393KB) keep single-engine
                            # transfer time short so arrivals stay smooth
                            weng = [nc.gpsimd, nc.sync,
                                    nc.scalar][(dt // 2) % 3]
                            weng.dma_start(
                                out=W[:, dt * NW:(dt + 2) * NW].rearrange(
                                    "p (a c) -> p a c", a=2),
                                in_=wqkv[dt * 128:(dt + 2) * 128, :].rearrange(
                                    "(a b) c -> b a c", b=128),
                            )
                        if dt % 2 == 0:
                            if sc == S_CHUNKS - 1:
                                # the last chunk's x tiles stay resident: the
                                # deferred Q projections in phase 2 reuse
                                # them, saving a 4MB re-fetch
                                xt = x3pool.tile([128, 1024], f16, tag="x3",
                                                 name=f"x3_{dt // 2}")
                                x3_tiles.append(xt)
                            elif sc == 0:
                                # sc0 is DMA-BW-bound: its own pairs get a
                                # full-depth pool (no recycle gating), while
                                # sc1's prefetch sits in a shallow pool so
                                # it can't steal sc0's scarce bandwidth
                                xt = x0pool.tile([128, 1024], f16, tag="x0",
                                                 name=f"x0_{dt // 2}")
                            else:
                                xt = xpool.tile([128, 1024], f16, tag="x",
                                                name=f"x_{sc}_{dt // 2}")
                            # never share the slot's W-issue engine
                            eng = nc.gpsimd if (dt // 2) % 3 == 1 else nc.sync
                            eng.dma_start(
                                out=xt[:].rearrange("p (a c) -> p a c", a=2),
                                in_=xT[dt * 128:(dt + 2) * 128,
                                       sc * 512:(sc + 1) * 512].rearrange(
                                    "(a b) c -> b a c", b=128),
                            )
                        xsl = xt[:, (dt % 2) * 512:(dt % 2 + 1) * 512]
                        for nt in nts:
                            if dt == 0:
                                psums[nt] = ppsum.tile(
                                    [128, 512], f32, tag="proj",
                                    name=f"proj_{sc}_{nt}")
                            nc.tensor.matmul(
                                psums[nt][:],
                                W[:, dt * NW + nt * 128:
                                  dt * NW + (nt + 1) * 128],
                                xsl,
                                start=(dt == 0),
                                stop=(dt == D_TILES - 1),
                            )
                    if sc == 0:
                        # constants are first needed by the RoPE below
                        nc.sync.dma_start(out=COS2[:], in_=cos2[:])
                        nc.sync.dma_start(out=SINM2[:], in_=sinm2[:])
                        nc.sync.dma_start(out=IDENT[:], in_=ident[:])

                    def copy_out(nt, use_act):
                        # the copies serialize on one engine and gate both
                        # PSUM-bank release and RoPE/transpose, so alternate
                        # them between ACT and DVE
                        dst = QKVT[:, nt * S + sc * 512:
                                   nt * S + (sc + 1) * 512]
                        if use_act:
                            nc.scalar.copy(dst, psums[nt][:])
                        else:
                            nc.vector.tensor_copy(dst, psums[nt][:])
                    # copies first: they free the PSUM banks the next
                    # s-chunk's first matmuls need
                    for ci, nt in enumerate(nts):
                        copy_out(nt, ci % 2 == 0)
                    rope(4, sc)
                    rope(0, sc)
                    for st in range(4 * sc, 4 * (sc + 1)):
                        transpose_v(st)
                    for h in [1, 2, 3]:
                        if h in nts:
                            rope(h, sc)

            if debug:
                dbg_rope = nc.dram_tensor(
                    "dbg_rope", [128, N_TILES * S], f32r,
                    kind="ExternalOutput").ap()
                dbg_vn = nc.dram_tensor(
                    "dbg_vn", [128, S], f32r, kind="ExternalOutput").ap()
                nc.sync.dma_start(out=dbg_rope[:], in_=QKVT[:])
                nc.sync.dma_start(out=dbg_vn[:], in_=VN[:])

            # -------- Phase 2+3: attention and output proj per q-chunk ------
            # normalized attn^T for head h overwrites QKVT's Q-head slice h
            # (each Q chunk is fully consumed by its own k-loop before the
            # in-place write)
            with (
                tc.tile_pool(name="wopool", bufs=1) as wopool,
                tc.tile_pool(name="spsum", bufs=3, space="PSUM") as spsum,
                tc.tile_pool(name="opsum", bufs=2, space="PSUM") as opsum,
                tc.tile_pool(name="smpsum", bufs=1, space="PSUM") as smpsum,
                tc.tile_pool(name="fpsum", bufs=2, space="PSUM") as fpsum,
                tc.tile_pool(name="expp", bufs=12) as expp,
                tc.tile_pool(name="accp", bufs=3) as accp,
                tc.tile_pool(name="rcp", bufs=2) as rcp,
                tc.tile_pool(name="fstage", bufs=4) as fstage,
            ):
                WO = wopool.tile([128, QH * DIM], f16, tag="wo")
                nc.gpsimd.dma_start(out=ONES[:], in_=ones[:])
                for ht in range(QH):
                    nc.sync.dma_start(
                        out=WO[:, ht * DIM:(ht + 1) * DIM],
                        in_=wo[ht * 128:(ht + 1) * 128, :],
                    )
                sc_last = S_CHUNKS - 1
                # x tiles of the last s-chunk are still resident from phase 1
                xt2s = x3_tiles

                def defer_steps():
                    # projection of Q1..Q3 for the last s-chunk, deferred
                    # into qc0's (ACT-paced) k-loops as PE filler. These
                    # slices are only read by (h, qc3) much later. The
                    # accumulators borrow the wo-projection's PSUM ring
                    # (tag "pf"), which is idle until wo_steps(qc0) runs.
                    for nt in [1, 2, 3]:
                        pr = fpsum.tile([128, 512], f32, tag="pf",
                                        name=f"defq_{nt}")
                        for dt in range(D_TILES):
                            nc.tensor.matmul(
                                pr[:],
                                W[:, dt * NW + nt * 128:
                                  dt * NW + (nt + 1) * 128],
                                xt2s[dt // 2][:, (dt % 2) * 512:
                                              (dt % 2 + 1) * 512],
                                start=(dt == 0), stop=(dt == D_TILES - 1),
                            )
                            yield
                        nc.vector.tensor_copy(
                            QKVT[:, nt * S + sc_last * 512:
                                 nt * S + (sc_last + 1) * 512],
                            pr[:])
                        rope(nt, sc_last)
                        yield

                def wo_steps(qc, drain=False):
                    # output projection for q-chunk qc, as a generator that
                    # yields after every matmul so the emitter can interleave
                    # these PE-only ops into the (ACT-paced) attention k-loops
                    # of the NEXT q-chunk, keeping the in-order PE dense
                    for qt in range(4 * qc, 4 * (qc + 1)):
                        for ec in range(E_CHUNKS):
                            pf = fpsum.tile([128, 512], f32, tag="pf",
                                            name=f"pf_{qt}_{ec}")
                            # head 3 LAST: the group's blocking operand is
                            # the most recently normalized head, so putting
                            # it last hides its norm chain behind the other
                            # three matmuls
                            hts = list(range(QH))
                            for j, ht in enumerate(hts):
                                nc.tensor.matmul(
                                    pf[:],
                                    QKVT[:, ht * S + qt * 128:
                                         ht * S + (qt + 1) * 128],
                                    WO[:, ht * DIM + ec * 512:
                                       ht * DIM + (ec + 1) * 512],
                                    start=(j == 0), stop=(j == QH - 1),
                                )
                                if j < QH - 1:
                                    yield
                            fs = fstage.tile([128, 512], f16, tag="fs",
                                             name=f"fs_{qt}_{ec}")
                            # ACT must stay exp-only here: routing any of
                            # these copies to it convoys the exp pipeline
                            # (+20us measured); gpsimd can't read PSUM
                            if drain and ec % 2 == 0:
                                nc.scalar.copy(fs[:], pf[:])
                            else:
                                nc.vector.tensor_copy(fs[:], pf[:])
                            nc.sync.dma_start(
                                out=outp[qt * 128:(qt + 1) * 128,
                                         ec * 512:(ec + 1) * 512],
                                in_=fs[:],
                            )
                            yield

                # [generator, remaining-yield-count]
                pending_wo = [defer_steps(), 3 * (D_TILES + 1)]
                pending_norm = None
                for qc in range(S_CHUNKS):
                    def norm_tail(nqc, h, po, acc):
                        base = h * S + nqc * 512
                        ps_sum = smpsum.tile([128, 512], f32, tag="psum",
                                             name=f"psum_{nqc}_{h}")
                        nc.tensor.matmul(ps_sum[:], ONES[:], acc[:],
                                         start=True, stop=True)
                        rc = rcp.tile([128, 512], f32, tag="rc",
                                      name=f"rc_{nqc}_{h}")
                        nc.vector.reciprocal_approx_fast(out=rc[:],
                                                         in_=ps_sum[:])
                        # 4 per-qt muls instead of one: the first wo group
                        # only reads qt-chunk 0, so subtile deps release it
                        # ~500ns earlier
                        for qt in range(4):
                            csl = slice(qt * 128, (qt + 1) * 128)
                            nc.vector.tensor_mul(
                                QKVT[:, base + qt * 128:
                                     base + (qt + 1) * 128],
                                po[:, csl], rc[:, csl])

                    # the norm chain (ones-matmul -> reciprocal -> mul) for
                    # head h is emitted at kt0 of the NEXT head's k-loop
                    # (crossing qc boundaries): emitted directly after its
                    # own k-loop, the in-order PE parks behind the DVE/ACT
                    # exp-accumulate chain while ready score matmuls wait
                    # behind it. Not in qc0 (h<3): defer_steps shares the
                    # smpsum bank mid-accumulation there.
                    for h in range(QH):
                        qsl = slice(h * S + qc * 512, h * S + (qc + 1) * 512)
                        po = opsum.tile([128, 512], f32, tag="po",
                                        name=f"po_{qc}_{h}")
                        acc = accp.tile([128, 512], f16, tag="acc",
                                        name=f"acc_{qc}_{h}")
                        et_first = None
                        for kt in range(S_TILES):
                            pscore = spsum.tile([128, 512], f32, tag="score",
                                                name=f"score_{qc}_{h}_{kt}")
                            nc.tensor.matmul(
                                pscore[:],
                                QKVT[:, 4 * S + kt * 128:
                                     4 * S + (kt + 1) * 128],
                                QKVT[:, qsl],
                                start=True, stop=True,
                            )
                            et = expp.tile([128, 512], f16, tag="exp",
                                           name=f"exp_{qc}_{h}_{kt}")
                            # constant shift keeps exp within fp16 range
                            # (max scaled score ~11.3 > ln(65504)); softmax
                            # is shift-invariant so it cancels exactly
                            nc.scalar.activation(
                                et[:], pscore[:],
                                mybir.ActivationFunctionType.Exp,
                                scale=SCALE, bias=EBIAS[:],
                            )
                            nc.tensor.matmul(
                                po[:],
                                VN[:, kt * 128:(kt + 1) * 128],
                                et[:],
                                start=(kt == 0), stop=(kt == S_TILES - 1),
                            )
                            if pending_norm is not None and (
                                    kt == 1 or pending_norm[1] == QH - 1):
                                # boundary norms (h3) emit at kt0 -- the
                                # next wo group needs them ASAP; mid-qc
                                # norms wait one kt so the exp/add tail
                                # they depend on is fully clear
                                norm_tail(*pending_norm)
                                pending_norm = None
                            # accumulate exp on DVE (keeps PE free); one
                            # all-ones matmul at the end both k-reduces and
                            # broadcasts the denominator to all partitions
                            if kt == 0:
                                et_first = et
                            elif kt == 1:
                                nc.vector.tensor_add(acc[:], et_first[:], et[:])
                            else:
                                nc.vector.tensor_add(acc[:], acc[:], et[:])
                            if pending_wo[1] > 0:
                                # spread the filler budget so no k-loop
                                # stretch is left fully unfilled (an empty
                                # stretch is ACT-paced: exp 810ns/kt vs the
                                # PE's own 426ns/kt)
                                kts_left = ((QH - 1 - h) * S_TILES
                                            + (S_TILES - 1 - kt))
                                # catch up with 3 pulls when behind so no
                                # leftover burst drains at the qc boundary
                                # (dense drains outrun the DVE staging
                                # copies that release the pf PSUM ring)
                                if qc > 0 and h == 0 and kt < 4:
                                    # 1/kt at the qc boundary: the first
                                    # group's h3 matmul then lands after
                                    # the norm chain it waits on
                                    take = 1
                                elif pending_wo[1] > 2 * kts_left:
                                    take = 3
                                elif pending_wo[1] > kts_left:
                                    take = 2
                                else:
                                    take = 1
                                for _ in range(min(take, pending_wo[1])):
                                    if next(pending_wo[0], None) is None:
                                        pending_wo[1] = 0
                                        break
                                    pending_wo[1] -= 1
                        if qc == S_CHUNKS - 1 and h == QH - 1:
                            norm_tail(qc, h, po, acc)
                        else:
                            pending_norm = (qc, h, po, acc)
                    for _ in pending_wo[0]:
                        pass
                    if qc < S_CHUNKS - 1:
                        pending_wo = [wo_steps(qc), 32 * 4]
                # last q-chunk's output projection runs dense at the end
                # (ACT is idle there, so staging copies go to it)
                for _ in wo_steps(S_CHUNKS - 1, drain=True):
                    pass

    nc.compile()
    return nc


def _prep_inputs(x, freqs_cos, freqs_sin, wq, wk, wv, wo):
    """Host-side layout prep. Returns per-core input maps."""
    x = np.asarray(x, np.float32)
    freqs_cos = np.asarray(freqs_cos, np.float32)
    freqs_sin = np.asarray(freqs_sin, np.float32)
    wq = np.asarray(wq, np.float32)
    wk = np.asarray(wk, np.float32)
    wv = np.asarray(wv, np.float32)
    wo = np.asarray(wo, np.float32)

    xT = np.ascontiguousarray(x.T.astype(np.float16))
    # stream_shuffle permutes single partitions within each 32-partition
    # block (same mask replicated across the 4 blocks), so lay out RoPE
    # pairs block-locally: block b, lane i<16 holds t1 of pair 16b+i
    # (even channel), lane 16+i holds t2 (odd channel). The swap mask
    # (i+16)%32 then exchanges t1/t2 within every block.
    perm = np.empty(HEAD_DIM, np.int64)
    fidx = np.empty(HEAD_DIM, np.int64)   # pair (frequency) index per row
    sgn = np.empty(HEAD_DIM, np.float32)  # sin sign per row
    for r in range(HEAD_DIM):
        b, i = divmod(r, 32)
        if i < 16:
            f = 16 * b + i
            perm[r], fidx[r], sgn[r] = 2 * f, f, -1.0
        else:
            f = 16 * b + (i - 16)
            perm[r], fidx[r], sgn[r] = 2 * f + 1, f, 1.0
    cosT = freqs_cos.T                      # (64, S)
    sinT = freqs_sin.T
    cos2 = np.ascontiguousarray(cosT[fidx, :].astype(np.float16))
    sinm2 = np.ascontiguousarray((sinT[fidx, :] * sgn[:, None]).astype(np.float16))
    ident = np.eye(128, dtype=np.float16)
    ones = np.ones((128, 128), np.float16)

    in_maps = []
    for c in range(N_CORES):
        wq_c = wq[:, c * QW:(c + 1) * QW].reshape(DIM, QH, HEAD_DIM)
        wq_c = np.ascontiguousarray(wq_c[:, :, perm].reshape(DIM, QW))
        wk_c = np.ascontiguousarray(
            wk[:, c * HEAD_DIM:(c + 1) * HEAD_DIM][:, perm])
        wv_c = wv[:, c * HEAD_DIM:(c + 1) * HEAD_DIM]
        wqkv_c = np.ascontiguousarray(
            np.concatenate([wq_c, wk_c, wv_c], axis=1).astype(np.float16))
        wo_c = np.ascontiguousarray(wo[c * QW:(c + 1) * QW, :].astype(np.float16))
        in_maps.append({
            "xT": xT, "wqkv": wqkv_c, "wo": wo_c,
            "cos2": cos2, "sinm2": sinm2, "ident": ident, "ones": ones,
        })
    return in_maps


def kernel(x, freqs_cos, freqs_sin, wq, wk, wv, wo):
    from concourse.bass_utils import run_bass_kernel_spmd

    if "nc" not in _cache:
        _cache["nc"] = _build_program()
    nc = _cache["nc"]

    in_maps = _prep_inputs(x, freqs_cos, freqs_sin, wq, wk, wv, wo)
    trace = bool(int(os.environ.get("KERNEL_TRACE", "0")))
    res = run_bass_kernel_spmd(
        nc, in_maps, list(range(N_CORES)), trace=trace,
    )
    _cache["last_result"] = res
    out = np.zeros((S, DIM), np.float32)
    for c in range(N_CORES):
        out += res.results[c]["outp"].astype(np.float32)
    return out



# revision 34
# speedup vs baseline: 1.0028x; 1.0028x over previous
"""Trainium2 Bass kernel for GQA attention (S=2048, D=4096, 32 Q heads / 8 KV
heads, head_dim=128, RoPE, full softmax) tensor-parallel over 8 NeuronCores.

Sharding: Megatron TP. Core c gets Q heads 4c..4c+3 and KV head c
(column-shard wq/wk/wv, row-shard wo). The output all-reduce is done on host
(sum of 8 partial outputs).

Layout: every on-device matmul contracts over the partition dim, so the
projection is computed with *transposed* output (channels on partitions) by
feeding x^T (host-prepped). RoPE pairs are laid out so DVE stream_shuffle
(which permutes lanes within each 32-partition block) swaps t1/t2, making the
rotation 4 elementwise DVE ops per chunk. scores^T (k on partitions, q free)
feeds exp on ACT; exp tiles feed the V-matmul (V natural layout via one PE
transpose per 128-row tile) and a ones-column matmul accumulating the softmax
denominator. attn^T overwrites the consumed Q slice and feeds the wo matmul
as the stationary operand with no transposes. Output projection is
interleaved per q-chunk with the attention of the next chunk.

Scheduling notes (trace-driven):
- The steady-state N=512 fp16 matmul issue gap is ~218ns (streaming bound
  512/2.4GHz + NX overhead); the kernel keeps the in-order PE within ~3% of
  that by never letting a slow engine's chain sit in front of ready matmuls.
- Softmax normalization (ones-matmul -> reciprocal -> per-qt muls) for each
  head is emitted at kt0 of the NEXT head's k-loop; emitted in place it
  parks the PE behind the ACT/DVE exp-accumulate tail (~2-3us per incident,
  plus a HAM re-throttle to 1.2GHz after any >3.4us idle).
- wo/deferred-Q matmuls are PE filler pulled into the (ACT-paced) attention
  k-loops on a budget scheduler (1-3 pulls/kt) so no stretch of the k-loop
  is left filler-less and no leftover burst drains at qc boundaries.
- ACT must stay exp-only in phase 2: routing even part of the PSUM->SBUF
  staging copies to it convoys the exp pipeline (+20us measured).
- sc0 of the projection is DMA-bandwidth-bound (~10.3MB before its matmuls
  can run densely, HBM saturated at ~340GB/s): W rides 2-dt 393KB batched
  3D-AP transfers over three issue queues, sc0's x pairs get a full-depth
  pool while sc1's prefetch sits in a shallow pool so it cannot steal sc0
  bandwidth; 16 junk warmup matmuls cover the first-arrival latency and
  hold the PE HAM clock at 8/8.
- fp16 output + fp32 host reduce; fp8 was evaluated numerically and is
  unusable here (one fp8 matmul already gives 3-9% rel err vs the 2e-2
  gate, since per-element quantization noise passes ~1:1 to the output).
"""

import os
import sys

sys.path.insert(0, "/opt/trn_rl_repo")

import numpy as np

S = 2048
DIM = 4096
N_HEADS = 32
N_KV_HEADS = 8
HEAD_DIM = 128
N_CORES = 8
QH = N_HEADS // N_CORES          # 4 Q heads per core
QW = QH * HEAD_DIM               # 512 Q cols per core
NW = QW + 2 * HEAD_DIM           # 768 projection cols per core (q + k + v)
SCALE = HEAD_DIM ** -0.5
D_TILES = DIM // 128             # 32
S_TILES = S // 128               # 16
S_CHUNKS = S // 512              # 4
N_TILES = NW // 128              # 6
E_CHUNKS = DIM // 512            # 8

_cache = {}


def _build_program(debug=False):
    import concourse.mybir as mybir
    import concourse.tile as tile
    from concourse import bacc

    f32 = mybir.dt.float32
    f32r = mybir.dt.float32r
    f16 = mybir.dt.float16

    nc = bacc.Bacc("TRN2", target_bir_lowering=False, debug=False)

    xT = nc.dram_tensor("xT", [DIM, S], f16, kind="ExternalInput").ap()
    wqkv = nc.dram_tensor("wqkv", [DIM, NW], f16, kind="ExternalInput").ap()
    wo = nc.dram_tensor("wo", [QW, DIM], f16, kind="ExternalInput").ap()
    cos2 = nc.dram_tensor("cos2", [128, S], f16, kind="ExternalInput").ap()
    sinm2 = nc.dram_tensor("sinm2", [128, S], f16, kind="ExternalInput").ap()
    ident = nc.dram_tensor("ident", [128, 128], f16, kind="ExternalInput").ap()
    ones = nc.dram_tensor("ones", [128, 128], f16, kind="ExternalInput").ap()
    # fp16 output halves the output-DMA bytes; the host all-reduce sums in
    # fp32 (adds ~2e-4 rel err vs the 2e-2 gate)
    outp = nc.dram_tensor("outp", [S, DIM], f16, kind="ExternalOutput").ap()

    # stream_shuffle: out lane (32b+i) = in lane (32b + mask[i])
    swap_mask = [(i + 16) % 32 for i in range(32)]

    with tile.TileContext(nc) as tc:
        with (
            tc.tile_pool(name="persist", bufs=1) as persist,
            tc.tile_pool(name="vn", bufs=1) as vnp,
            tc.tile_pool(name="wpool", bufs=1) as wpool,
            tc.tile_pool(name="cspool", bufs=1) as cspool,
            tc.tile_pool(name="rtmp", bufs=2) as rtmp,
            tc.tile_pool(name="x3pool", bufs=D_TILES // 2) as x3pool,
        ):
            QKVT = persist.tile([128, N_TILES * S], f16, tag="qkvt")
            EBIAS = persist.tile([128, 1], f32, tag="ebias")
            nc.gpsimd.memset(EBIAS[:], -4.0)
            VN = vnp.tile([128, S], f16, tag="vn")
            ONES = persist.tile([128, 128], f16, tag="ones")

            W = wpool.tile([128, D_TILES * NW], f16, tag="w")
            COS2 = cspool.tile([128, S], f16, tag="cos2")
            SINM2 = cspool.tile([128, S], f16, tag="sinm2")
            IDENT = cspool.tile([128, 128], f16, tag="ident")

            def rope(h, sc):
                sl = slice(h * S + sc * 512, h * S + (sc + 1) * 512)
                csl = slice(sc * 512, (sc + 1) * 512)
                sh = rtmp.tile([128, 512], f16, tag="sh",
                               name=f"sh_{h}_{sc}")
                nc.vector.stream_shuffle(sh[:], QKVT[:, sl], swap_mask)
                nc.vector.tensor_mul(sh[:], sh[:], SINM2[:, csl])
                tm = rtmp.tile([128, 512], f16, tag="tm",
                               name=f"tm_{h}_{sc}")
                nc.vector.tensor_mul(tm[:], QKVT[:, sl], COS2[:, csl])
                nc.vector.tensor_add(QKVT[:, sl], sh[:], tm[:])

            # ---------------- Phase 1: fused QKV projection (transposed) ---
            with (
                tc.tile_pool(name="xpool", bufs=4) as xpool,
                tc.tile_pool(name="x0pool", bufs=16) as x0pool,
                tc.tile_pool(name="ppsum", bufs=7, space="PSUM") as ppsum,
                tc.tile_pool(name="tpsum", bufs=1, space="PSUM") as tpsum,
            ):

                # K (nt=4) and V (nt=5) first so RoPE(K)/transpose(V) start
                # early
                nt_order = [4, 5, 0, 1, 2, 3]

                # HAM warmup: the first real matmul can't start until its
                # W/x DMAs land (~11us); run junk matmuls on a memset scratch
                # region meanwhile so the PE clock is at 8/8 when real work
                # arrives (else the first ~3.4us run at half rate).
                # gpsimd clears its preamble ~1.4us before DVE does, so the
                # memset (and with it the warmup) starts that much earlier
                nc.gpsimd.memset(QKVT[:, 0:640], 0.0)
                wpt = ppsum.tile([128, 512], f32, tag="proj", name="warm")
                for _ in range(16):
                    nc.tensor.matmul(wpt[:], QKVT[:, 512:640],
                                     QKVT[:, 0:512],
                                     start=True, stop=True)

                def transpose_v(st):
                    pt = tpsum.tile([128, 128], f16, tag="vt",
                                    name=f"vt_{st}")
                    nc.tensor.transpose(
                        pt[:],
                        QKVT[:, 5 * S + st * 128: 5 * S + (st + 1) * 128],
                        IDENT[:],
                    )
                    nc.scalar.copy(VN[:, st * 128:(st + 1) * 128], pt[:])

                # each dma_start costs ~600ns of sequencer issue time, so
                # batch transfers: W rides in 4-dt groups (786KB contiguous
                # 1536B lines), x in 2-dt pairs (256KB) — halves the issue
                # count for sc=0, which must stream W 6.3MB + x 4MB before
                # its matmuls can run densely
                x3_tiles = []
                for sc in range(S_CHUNKS):
                    nts = nt_order if sc < S_CHUNKS - 1 else [4, 5, 0]
                    psums = {}
                    for dt in range(D_TILES):
                        if sc == 0 and dt % 2 == 0:
                            # three issue engines for sc0's W burst (ACT's
                            # DMAs finish long before its first copy_out);
                            # 2-dt groups (393KB) keep single-engine
                            # transfer time short so arrivals stay smooth
                            weng = [nc.gpsimd, nc.sync,
                                    nc.scalar][(dt // 2) % 3]
                            weng.dma_start(
                                out=W[:, dt * NW:(dt + 2) * NW].rearrange(
                                    "p (a c) -> p a c", a=2),
                                in_=wqkv[dt * 128:(dt + 2) * 128, :].rearrange(
                                    "(a b) c -> b a c", b=128),
                            )
                        if dt % 2 == 0:
                            if sc == S_CHUNKS - 1:
                                # the last chunk's x tiles stay resident: the
                                # deferred Q projections in phase 2 reuse
                                # them, saving a 4MB re-fetch
                                xt = x3pool.tile([128, 1024], f16, tag="x3",
                                                 name=f"x3_{dt // 2}")
                                x3_tiles.append(xt)
                            elif sc == 0:
                                # sc0 is DMA-BW-bound: its own pairs get a
                                # full-depth pool (no recycle gating), while
                                # sc1's prefetch sits in a shallow pool so
                                # it can't steal sc0's scarce bandwidth
                                xt = x0pool.tile([128, 1024], f16, tag="x0",
                                                 name=f"x0_{dt // 2}")
                            else:
                                xt = xpool.tile([128, 1024], f16, tag="x",
                                                name=f"x_{sc}_{dt // 2}")
                            # never share the slot's W-issue engine
                            eng = nc.gpsimd if (dt // 2) % 3 == 1 else nc.sync
                            eng.dma_start(
                                out=xt[:].rearrange("p (a c) -> p a c", a=2),
                                in_=xT[dt * 128:(dt + 2) * 128,
                                       sc * 512:(sc + 1) * 512].rearrange(
                                    "(a b) c -> b a c", b=128),
                            )
                        xsl = xt[:, (dt % 2) * 512:(dt % 2 + 1) * 512]
                        for nt in nts:
                            if dt == 0:
                                psums[nt] = ppsum.tile(
                                    [128, 512], f32, tag="proj",
                                    name=f"proj_{sc}_{nt}")
                            nc.tensor.matmul(
                                psums[nt][:],
                                W[:, dt * NW + nt * 128:
                                  dt * NW + (nt + 1) * 128],
                                xsl,
                                start=(dt == 0),
                                stop=(dt == D_TILES - 1),
                            )
                        if sc == 0 and 2 <= dt <= 16 and dt % 2 == 0:
                            # sc0's W/x arrivals are bursty at saturated
                            # HBM; a couple of junk matmuls per even dt
                            # keep the PE HAM clock at 8/8 through the
                            # arrival stalls (the warm bank isn't recycled
                            # until sc1's first psum allocation)
                            for _ in range(2):
                                nc.tensor.matmul(wpt[:], QKVT[:, 512:640],
                                                 QKVT[:, 0:512],
                                                 start=True, stop=True)
                    if sc == 0:
                        # constants are first needed by the RoPE below
                        nc.sync.dma_start(out=COS2[:], in_=cos2[:])
                        nc.sync.dma_start(out=SINM2[:], in_=sinm2[:])
                        nc.sync.dma_start(out=IDENT[:], in_=ident[:])

                    def copy_out(nt, use_act):
                        # the copies serialize on one engine and gate both
                        # PSUM-bank release and RoPE/transpose, so alternate
                        # them between ACT and DVE
                        dst = QKVT[:, nt * S + sc * 512:
                                   nt * S + (sc + 1) * 512]
                        if use_act:
                            nc.scalar.copy(dst, psums[nt][:])
                        else:
                            nc.vector.tensor_copy(dst, psums[nt][:])
                    # copies first: they free the PSUM banks the next
                    # s-chunk's first matmuls need
                    # at sc3, V first (feeds the transposes), Q0 next
                    # (its bank becomes the first score psum), K last (its
                    # RoPE isn't consumed until kt12 of qc0, ~14us later)
                    corder = [5, 0, 4] if sc == S_CHUNKS - 1 else nts
                    for ci, nt in enumerate(corder):
                        copy_out(nt, ci % 2 == 0)
                    rope(4, sc)
                    rope(0, sc)
                    for st in range(4 * sc, 4 * (sc + 1)):
                        transpose_v(st)
                    for h in [1, 2, 3]:
                        if h in nts:
                            rope(h, sc)

            if debug:
                dbg_rope = nc.dram_tensor(
                    "dbg_rope", [128, N_TILES * S], f32r,
                    kind="ExternalOutput").ap()
                dbg_vn = nc.dram_tensor(
                    "dbg_vn", [128, S], f32r, kind="ExternalOutput").ap()
                nc.sync.dma_start(out=dbg_rope[:], in_=QKVT[:])
                nc.sync.dma_start(out=dbg_vn[:], in_=VN[:])

            # -------- Phase 2+3: attention and output proj per q-chunk ------
            # normalized attn^T for head h overwrites QKVT's Q-head slice h
            # (each Q chunk is fully consumed by its own k-loop before the
            # in-place write)
            with (
                tc.tile_pool(name="wopool", bufs=1) as wopool,
                tc.tile_pool(name="spsum", bufs=3, space="PSUM") as spsum,
                tc.tile_pool(name="opsum", bufs=2, space="PSUM") as opsum,
                tc.tile_pool(name="smpsum", bufs=1, space="PSUM") as smpsum,
                tc.tile_pool(name="fpsum", bufs=2, space="PSUM") as fpsum,
                tc.tile_pool(name="expp", bufs=12) as expp,
                tc.tile_pool(name="accp", bufs=3) as accp,
                tc.tile_pool(name="rcp", bufs=2) as rcp,
                tc.tile_pool(name="fstage", bufs=4) as fstage,
            ):
                WO = wopool.tile([128, QH * DIM], f16, tag="wo")
                nc.gpsimd.dma_start(out=ONES[:], in_=ones[:])
                for ht in range(QH):
                    nc.sync.dma_start(
                        out=WO[:, ht * DIM:(ht + 1) * DIM],
                        in_=wo[ht * 128:(ht + 1) * 128, :],
                    )
                sc_last = S_CHUNKS - 1
                # x tiles of the last s-chunk are still resident from phase 1
                xt2s = x3_tiles

                def defer_steps():
                    # projection of Q1..Q3 for the last s-chunk, deferred
                    # into qc0's (ACT-paced) k-loops as PE filler. These
                    # slices are only read by (h, qc3) much later. The
                    # accumulators borrow the wo-projection's PSUM ring
                    # (tag "pf"), which is idle until wo_steps(qc0) runs.
                    for nt in [1, 2, 3]:
                        pr = fpsum.tile([128, 512], f32, tag="pf",
                                        name=f"defq_{nt}")
                        for dt in range(D_TILES):
                            nc.tensor.matmul(
                                pr[:],
                                W[:, dt * NW + nt * 128:
                                  dt * NW + (nt + 1) * 128],
                                xt2s[dt // 2][:, (dt % 2) * 512:
                                              (dt % 2 + 1) * 512],
                                start=(dt == 0), stop=(dt == D_TILES - 1),
                            )
                            yield
                        nc.vector.tensor_copy(
                            QKVT[:, nt * S + sc_last * 512:
                                 nt * S + (sc_last + 1) * 512],
                            pr[:])
                        rope(nt, sc_last)
                        yield

                def wo_steps(qc, drain=False):
                    # output projection for q-chunk qc, as a generator that
                    # yields after every matmul so the emitter can interleave
                    # these PE-only ops into the (ACT-paced) attention k-loops
                    # of the NEXT q-chunk, keeping the in-order PE dense
                    for qt in range(4 * qc, 4 * (qc + 1)):
                        for ec in range(E_CHUNKS):
                            pf = fpsum.tile([128, 512], f32, tag="pf",
                                            name=f"pf_{qt}_{ec}")
                            # head 3 LAST: the group's blocking operand is
                            # the most recently normalized head, so putting
                            # it last hides its norm chain behind the other
                            # three matmuls
                            hts = list(range(QH))
                            for j, ht in enumerate(hts):
                                nc.tensor.matmul(
                                    pf[:],
                                    QKVT[:, ht * S + qt * 128:
                                         ht * S + (qt + 1) * 128],
                                    WO[:, ht * DIM + ec * 512:
                                       ht * DIM + (ec + 1) * 512],
                                    start=(j == 0), stop=(j == QH - 1),
                                )
                                if j < QH - 1:
                                    yield
                            fs = fstage.tile([128, 512], f16, tag="fs",
                                             name=f"fs_{qt}_{ec}")
                            # ACT must stay exp-only here: routing any of
                            # these copies to it convoys the exp pipeline
                            # (+20us measured); gpsimd can't read PSUM
                            if drain and ec % 2 == 0:
                                nc.scalar.copy(fs[:], pf[:])
                            else:
                                nc.vector.tensor_copy(fs[:], pf[:])
                            nc.sync.dma_start(
                                out=outp[qt * 128:(qt + 1) * 128,
                                         ec * 512:(ec + 1) * 512],
                                in_=fs[:],
                            )
                            yield

                # [generator, remaining-yield-count]
                pending_wo = [defer_steps(), 3 * (D_TILES + 1)]
                pending_norm = None
                for qc in range(S_CHUNKS):
                    def norm_tail(nqc, h, po, acc):
                        base = h * S + nqc * 512
                        ps_sum = smpsum.tile([128, 512], f32, tag="psum",
                                             name=f"psum_{nqc}_{h}")
                        nc.tensor.matmul(ps_sum[:], ONES[:], acc[:],
                                         start=True, stop=True)
                        rc = rcp.tile([128, 512], f32, tag="rc",
                                      name=f"rc_{nqc}_{h}")
                        nc.vector.reciprocal_approx_fast(out=rc[:],
                                                         in_=ps_sum[:])
                        # 4 per-qt muls instead of one: the first wo group
                        # only reads qt-chunk 0, so subtile deps release it
                        # ~500ns earlier
                        for qt in range(4):
                            csl = slice(qt * 128, (qt + 1) * 128)
                            nc.vector.tensor_mul(
                                QKVT[:, base + qt * 128:
                                     base + (qt + 1) * 128],
                                po[:, csl], rc[:, csl])

                    # the norm chain (ones-matmul -> reciprocal -> mul) for
                    # head h is emitted at kt0 of the NEXT head's k-loop
                    # (crossing qc boundaries): emitted directly after its
                    # own k-loop, the in-order PE parks behind the DVE/ACT
                    # exp-accumulate chain while ready score matmuls wait
                    # behind it. Not in qc0 (h<3): defer_steps shares the
                    # smpsum bank mid-accumulation there.
                    for h in range(QH):
                        qsl = slice(h * S + qc * 512, h * S + (qc + 1) * 512)
                        po = opsum.tile([128, 512], f32, tag="po",
                                        name=f"po_{qc}_{h}")
                        acc = accp.tile([128, 512], f16, tag="acc",
                                        name=f"acc_{qc}_{h}")
                        et_first = None
                        for kt in range(S_TILES):
                            pscore = spsum.tile([128, 512], f32, tag="score",
                                                name=f"score_{qc}_{h}_{kt}")
                            nc.tensor.matmul(
                                pscore[:],
                                QKVT[:, 4 * S + kt * 128:
                                     4 * S + (kt + 1) * 128],
                                QKVT[:, qsl],
                                start=True, stop=True,
                            )
                            et = expp.tile([128, 512], f16, tag="exp",
                                           name=f"exp_{qc}_{h}_{kt}")
                            # constant shift keeps exp within fp16 range
                            # (max scaled score ~11.3 > ln(65504)); softmax
                            # is shift-invariant so it cancels exactly
                            nc.scalar.activation(
                                et[:], pscore[:],
                                mybir.ActivationFunctionType.Exp,
                                scale=SCALE, bias=EBIAS[:],
                            )
                            nc.tensor.matmul(
                                po[:],
                                VN[:, kt * 128:(kt + 1) * 128],
                                et[:],
                                start=(kt == 0), stop=(kt == S_TILES - 1),
                            )
                            if pending_norm is not None and (
                                    kt == 1 or pending_norm[1] == QH - 1):
                                # boundary norms (h3) emit at kt0 -- the
                                # next wo group needs them ASAP; mid-qc
                                # norms wait one kt so the exp/add tail
                                # they depend on is fully clear
                                norm_tail(*pending_norm)
                                pending_norm = None
                            # accumulate exp on DVE (keeps PE free); one
                            # all-ones matmul at the end both k-reduces and
                            # broadcasts the denominator to all partitions
                            if kt == 0:
                                et_first = et
                            elif kt == 1:
                                nc.vector.tensor_add(acc[:], et_first[:], et[:])
                            else:
                                nc.vector.tensor_add(acc[:], acc[:], et[:])
                            if pending_wo[1] > 0:
                                # spread the filler budget so no k-loop
                                # stretch is left fully unfilled (an empty
                                # stretch is ACT-paced: exp 810ns/kt vs the
                                # PE's own 426ns/kt)
                                kts_left = ((QH - 1 - h) * S_TILES
                                            + (S_TILES - 1 - kt))
                                # catch up with 3 pulls when behind so no
                                # leftover burst drains at the qc boundary
                                # (dense drains outrun the DVE staging
                                # copies that release the pf PSUM ring)
                                if qc > 0 and h == 0 and kt < 4:
                                    # 1/kt at the qc boundary: the first
                                    # group's h3 matmul then lands after
                                    # the norm chain it waits on
                                    take = 1
                                elif pending_wo[1] > 2 * kts_left:
                                    take = 3
                                elif pending_wo[1] > kts_left:
                                    take = 2
                                else:
                                    take = 1
                                for _ in range(min(take, pending_wo[1])):
                                    if next(pending_wo[0], None) is None:
                                        pending_wo[1] = 0
                                        break
                                    pending_wo[1] -= 1
                        if qc == S_CHUNKS - 1 and h == QH - 1:
                            norm_tail(qc, h, po, acc)
                        else:
                            pending_norm = (qc, h, po, acc)
                    for _ in pending_wo[0]:
                        pass
                    if qc < S_CHUNKS - 1:
                        pending_wo = [wo_steps(qc), 32 * 4]
                # last q-chunk's output projection runs dense at the end
                # (ACT is idle there, so staging copies go to it)
                for _ in wo_steps(S_CHUNKS - 1, drain=True):
                    pass

    nc.compile()
    return nc


def _prep_inputs(x, freqs_cos, freqs_sin, wq, wk, wv, wo):
    """Host-side layout prep. Returns per-core input maps."""
    x = np.asarray(x, np.float32)
    freqs_cos = np.asarray(freqs_cos, np.float32)
    freqs_sin = np.asarray(freqs_sin, np.float32)
    wq = np.asarray(wq, np.float32)
    wk = np.asarray(wk, np.float32)
    wv = np.asarray(wv, np.float32)
    wo = np.asarray(wo, np.float32)

    xT = np.ascontiguousarray(x.T.astype(np.float16))
    # stream_shuffle permutes single partitions within each 32-partition
    # block (same mask replicated across the 4 blocks), so lay out RoPE
    # pairs block-locally: block b, lane i<16 holds t1 of pair 16b+i
    # (even channel), lane 16+i holds t2 (odd channel). The swap mask
    # (i+16)%32 then exchanges t1/t2 within every block.
    perm = np.empty(HEAD_DIM, np.int64)
    fidx = np.empty(HEAD_DIM, np.int64)   # pair (frequency) index per row
    sgn = np.empty(HEAD_DIM, np.float32)  # sin sign per row
    for r in range(HEAD_DIM):
        b, i = divmod(r, 32)
        if i < 16:
            f = 16 * b + i
            perm[r], fidx[r], sgn[r] = 2 * f, f, -1.0
        else:
            f = 16 * b + (i - 16)
            perm[r], fidx[r], sgn[r] = 2 * f + 1, f, 1.0
    cosT = freqs_cos.T                      # (64, S)
    sinT = freqs_sin.T
    cos2 = np.ascontiguousarray(cosT[fidx, :].astype(np.float16))
    sinm2 = np.ascontiguousarray((sinT[fidx, :] * sgn[:, None]).astype(np.float16))
    ident = np.eye(128, dtype=np.float16)
    ones = np.ones((128, 128), np.float16)

    in_maps = []
    for c in range(N_CORES):
        wq_c = wq[:, c * QW:(c + 1) * QW].reshape(DIM, QH, HEAD_DIM)
        wq_c = np.ascontiguousarray(wq_c[:, :, perm].reshape(DIM, QW))
        wk_c = np.ascontiguousarray(
            wk[:, c * HEAD_DIM:(c + 1) * HEAD_DIM][:, perm])
        wv_c = wv[:, c * HEAD_DIM:(c + 1) * HEAD_DIM]
        wqkv_c = np.ascontiguousarray(
            np.concatenate([wq_c, wk_c, wv_c], axis=1).astype(np.float16))
        wo_c = np.ascontiguousarray(wo[c * QW:(c + 1) * QW, :].astype(np.float16))
        in_maps.append({
            "xT": xT, "wqkv": wqkv_c, "wo": wo_c,
            "cos2": cos2, "sinm2": sinm2, "ident": ident, "ones": ones,
        })
    return in_maps


def kernel(x, freqs_cos, freqs_sin, wq, wk, wv, wo):
    from concourse.bass_utils import run_bass_kernel_spmd

    if "nc" not in _cache:
        _cache["nc"] = _build_program()
    nc = _cache["nc"]

    in_maps = _prep_inputs(x, freqs_cos, freqs_sin, wq, wk, wv, wo)
    trace = bool(int(os.environ.get("KERNEL_TRACE", "0")))
    res = run_bass_kernel_spmd(
        nc, in_maps, list(range(N_CORES)), trace=trace,
    )
    _cache["last_result"] = res
    out = np.zeros((S, DIM), np.float32)
    for c in range(N_CORES):
        out += res.results[c]["outp"].astype(np.float32)
    return out

